# revision 29
# baseline (speedup 1.0000x reference)
"""Trainium2 Bass kernel for the DActor dense MLP.

Network (per row of `state`):
    h1 = relu(state @ W1 + b1)        # 512 -> 500
    h2 = relu(h1 @ W2 + b2)           # 500 -> 300
    h3 = relu(h2 @ W3 + b3)           # 300 -> 100
    v  = h3 @ W4 + b4                 # 100 -> 64
    t  = tanh(v[:, :63]); s = sigmoid(v[:, 63:])
    possum = sum(relu(t)); denom = possum == 0 ? 1 : possum
    out = concat(where(t > 0, t / denom, t), s)

Strategy: pure data parallel over 8 NeuronCores (8192 rows each).
Activations are kept feature-major ([feat, batch]) for L1-L3 so every
matmul uses the natural [fan_in, fan_out] weight tile as the stationary
operand and the activations as the 512-wide moving operand. Everything
runs in bf16 (matmul rate equals fp32r on the PE, but weight loads get
the fast-weight-load path and SBUF/DMA traffic halves; fp32 PSUM
accumulate keeps end-to-end error ~1e-3, well inside the 2e-2 gate.
fp8 DoubleRow was emulated end-to-end at 1.5-2.8e-2 — too close to or
over the gate — and rejected).

Biases for L2/L3/L4 ride inside the weight matmuls through constant-1
activation rows placed in the zero padding (h1[500]=1 with W2 row 500
= b2, h2[300]=1 with W3 row 300 = b3, h3[100]=1 with W4 row 100 = b4).
L4 is computed batch-major: each 128-row h3 block is the *stationary*
operand and W4 the 64-wide moving operand, so PSUM directly holds
[batch, 64] and no PE transpose / PSUM->SBUF copy is needed.

Edge-time details: ~32 short warmup matmuls on a zero tile run while
the initial DMAs land, so the PE's HAM clock gate reaches 2.4 GHz
before the first real matmul; initial weight/x DMAs are spread across
four engine queues (each dma_start costs its issuing engine ~0.7us of
descriptor generation, so x loads go to sync/gpsimd, stores to vector,
and the scalar engine keeps only ACT work); L3's 45-row K-tail is
padded to 128 partitions with zeros (sub-128-partition matmuls measure
~100ns slower and delay their successor); the PST epilogue runs
per-chunk and the last chunk's store is split four ways to shrink the
end-of-kernel DMA tail.
"""

import numpy as np

import concourse.bass as bass
import concourse.tile as tile
from concourse import bacc, mybir
from concourse.bass_utils import run_bass_kernel_spmd

N_CORES = 8
BATCH = 65536
B = BATCH // N_CORES  # 8192 rows per core
D_IN, H1, H2, H3, D_OUT = 512, 500, 300, 100, 64
NCHUNK = 512  # moving-operand width (= 1 PSUM bank of fp32)
N_CHUNKS = B // NCHUNK  # 16
G = 4  # 128-row blocks per chunk = per batch-major PSUM tile

F32 = mybir.dt.float32
BF16 = mybir.dt.bfloat16

K1, K2, K3 = 4, 4, 3  # 128-row K tiles per layer (L2 K=501 incl bias row)
M1, M2 = 4, 3  # output 128-tiles for L1 (500->512) / L2 (300->384)
K4 = 101  # L4 contraction: 100 features + const row


def _emit(tc: tile.TileContext, aps: dict):
    nc = tc.nc
    xT = aps["xT"]
    out = aps["out"]

    consts = tc.alloc_tile_pool(name="consts", bufs=1)
    acts = tc.alloc_tile_pool(name="acts", bufs=3)
    outs = tc.alloc_tile_pool(name="outs", bufs=3)
    scratch = tc.alloc_tile_pool(name="scratch", bufs=2)
    psum_mm = tc.alloc_tile_pool(name="psum_mm", bufs=7, space="PSUM")
    psum_bm = tc.alloc_tile_pool(name="psum_bm", bufs=1, space="PSUM")

    xT_v = xT.rearrange("(k p) b -> p k b", p=128)  # [128, 4, B]
    # out rows = 128*g + p  ->  [p, g, f] (g = global 128-row block index)
    out_blocks = out.rearrange("(g p) f -> p g f", p=128)

    # ---- PE warmup: short matmuls on a zeroed tile while DMAs land ------
    wtmp = consts.tile([128, 128], BF16)
    nc.gpsimd.memset(wtmp, 0)
    psw = psum_mm.tile([128, NCHUNK], F32, tag="ps")  # shares the ps ring
    # Dummy tanh first: pulls the ACT table load (~1.3us, sigmoid set
    # also serves relu/sigmoid) into the startup DMA window instead of
    # stalling the first mid-stream PST.
    tldw = scratch.tile([128, 8], F32, tag="tldw")
    nc.scalar.activation(out=tldw, in_=wtmp[:, 0:8],
                         func=mybir.ActivationFunctionType.Tanh)

    # ---- persistent constants -------------------------------------------
    # Host-padded weights, bf16. W2/W3/W4 carry their bias in the constant
    # row (500/300/100); padded rows/cols are zero.
    w1 = consts.tile([128, K1, 512], BF16)
    w2 = consts.tile([128, K2, 384], BF16)
    w3 = consts.tile([128, K3, 128], BF16)
    w4 = consts.tile([128, D_OUT], BF16)
    b1 = consts.tile([128, M1], F32)
    bc2 = consts.tile([128, 1], F32)
    bc3 = consts.tile([128, 1], F32)

    # Only sync (SP), gpsimd, and scalar queues can issue DMAs; keep the
    # scalar engine's queue for startup only (ACT is the second-busiest).
    # Startup DMAs are ordered so w1-k0 and x0-k0 land first on separate
    # queues (they gate the first matmul; each trigger costs its engine
    # ~0.7us of descriptor generation).
    w1_v = aps["W1"].rearrange("(k p) m -> p k m", p=128)
    x_tiles = {}

    def load_x(c, engines):
        t = acts.tile([128, K1, NCHUNK], BF16, tag="x")
        cs = slice(c * NCHUNK, (c + 1) * NCHUNK)
        for ki in range(K1):
            engines[ki % len(engines)].dma_start(out=t[:, ki, :],
                                                 in_=xT_v[:, ki, cs])
        x_tiles[c] = t

    # Chunk 0 is processed as two 256-col halves (see `specs` below), and
    # its x/w1-k0 loads arrive as 256-col pieces on separate queues, so
    # the first matmul is gated by two 64KB transfers instead of 256KB.
    # All pieces keep the full 128-partition shape (64-partition DMAs and
    # large per-partition-contiguous runs both flip the chip into a 5/6
    # clock state -- measured, mechanism unknown).
    x0 = acts.tile([128, K1, NCHUNK], BF16, tag="x")
    x_tiles[0] = x0
    A, Bc = slice(0, 256), slice(256, 512)
    nc.sync.dma_start(out=w1[:, 0, A], in_=w1_v[:, 0, A])
    nc.gpsimd.dma_start(out=x0[:, 0, A], in_=xT_v[:, 0, A])
    nc.scalar.dma_start(out=w1[:, 0, Bc], in_=w1_v[:, 0, Bc])
    nc.sync.dma_start(out=x0[:, 1, A], in_=xT_v[:, 1, A])
    nc.gpsimd.dma_start(out=b1, in_=aps["b1"].rearrange("(m p) -> p m", p=128))
    nc.scalar.dma_start(out=x0[:, 1, Bc], in_=xT_v[:, 1, Bc])
    nc.sync.dma_start(out=w1[:, 2, :], in_=w1_v[:, 2, :])
    nc.gpsimd.dma_start(out=w1[:, 1, :], in_=w1_v[:, 1, :])
    nc.scalar.dma_start(out=w1[:, 3, :], in_=w1_v[:, 3, :])
    nc.sync.dma_start(out=x0[:, 3, A], in_=xT_v[:, 3, A])
    nc.gpsimd.dma_start(out=x0[:, 2, A], in_=xT_v[:, 2, A])
    nc.scalar.dma_start(out=x0[:, 3, Bc], in_=xT_v[:, 3, Bc])
    nc.gpsimd.dma_start(out=x0[:, 0, Bc], in_=xT_v[:, 0, Bc])
    nc.gpsimd.dma_start(out=x0[:, 2, Bc], in_=xT_v[:, 2, Bc])

    for _ in range(12):
        nc.tensor.matmul(psw[:, 0:128], wtmp, wtmp, start=True, stop=True)

    w2_v = aps["W2"].rearrange("(k p) m -> p k m", p=128)
    nc.sync.dma_start(out=w2[:, 0:2, :], in_=w2_v[:, 0:2, :])
    nc.scalar.dma_start(out=w2[:, 2:4, :], in_=w2_v[:, 2:4, :])
    nc.gpsimd.dma_start(out=bc2, in_=aps["bc2"].rearrange("(m o) -> m o", o=1))
    nc.gpsimd.dma_start(out=bc3, in_=aps["bc3"].rearrange("(m o) -> m o", o=1))
    nc.scalar.dma_start(out=w3, in_=aps["W3"].rearrange("(k p) m -> p k m", p=128))
    nc.gpsimd.dma_start(out=w4, in_=aps["W4"])
    load_x(1, [nc.sync, nc.gpsimd, nc.sync, nc.gpsimd])

    Relu = mybir.ActivationFunctionType.Relu
    prev = None  # (h3 tile, G blocks, global block base, store engine parity)

    # First and last 512-col chunks are processed as two 256-col halves:
    # the first so compute starts as soon as 64KB slices land, the last
    # so the end-of-kernel serial PST chain and final store are half-sized.
    specs = [(0, 0, 256), (0, 256, 256)]
    specs += [(c, 0, NCHUNK) for c in range(1, N_CHUNKS - 1)]
    specs += [(N_CHUNKS - 1, 0, 256), (N_CHUNKS - 1, 256, 256)]

    def emit_l4(is_last):
        # L4 for the previous chunk, batch-major: each 128-row h3 block is
        # the stationary operand, W4 (64 cols, bias in row 100) the moving
        # one. Emitted after the next chunk's L1 matmuls so the PE never
        # waits on the ACT-produced h3.
        if prev is None:
            return
        h3p, Gp, g0, par = prev
        bm = psum_bm.tile([128, Gp, D_OUT], F32, tag="bm")
        for bb in range(Gp):
            nc.tensor.matmul(
                bm[:, bb, :],
                h3p[:K4, bb * 128:(bb + 1) * 128],
                w4[:K4, :],
                start=True, stop=True,
            )
        _pst_store(nc, scratch, outs, bm, out_blocks, g0, Gp,
                   split_store=is_last,
                   store_eng=nc.sync if par == 0 else nc.gpsimd)

    for i, (c, off, w) in enumerate(specs):
        if c + 2 < N_CHUNKS and off == 0:
            load_x(c + 2, [nc.sync, nc.gpsimd, nc.sync, nc.gpsimd])
        x_sb = x_tiles[c]
        if off + w == NCHUNK:
            x_tiles.pop(c)
        ws = slice(off, off + w)

        # ---- layer 1: [512 -> 500(pad 512)], bias via ACT ---------------
        # m-tile 3's relu+bias runs on DVE (in parallel with ACT doing
        # m2) so h1-m3 is ready before L2's k3 matmul needs it.
        h1 = acts.tile([128, K2, NCHUNK], BF16, tag="h1")
        for mi in range(M1):
            ps = psum_mm.tile([128, NCHUNK], F32, tag="ps")
            msl = slice(mi * 128, (mi + 1) * 128)
            for ki in range(K1):
                nc.tensor.matmul(ps[:, 0:w], w1[:, ki, msl], x_sb[:, ki, ws],
                                 start=(ki == 0), stop=(ki == K1 - 1))
            if mi == M1 - 1:
                nc.vector.tensor_scalar(out=h1[:, mi, 0:w], in0=ps[:, 0:w],
                                        scalar1=b1[:, mi:mi + 1], scalar2=0.0,
                                        op0=mybir.AluOpType.add,
                                        op1=mybir.AluOpType.max)
            else:
                nc.scalar.activation(out=h1[:, mi, 0:w], in_=ps[:, 0:w],
                                     func=Relu, bias=b1[:, mi:mi + 1])
        emit_l4(False)

        # ---- layer 2: [501 -> 300(pad 384)], bias via W2 row 500 --------
        h2 = acts.tile([128, K3, NCHUNK], BF16, tag="h2")
        for mi in range(M2):
            ps = psum_mm.tile([128, NCHUNK], F32, tag="ps")
            msl = slice(mi * 128, (mi + 1) * 128)
            for ki in range(K2):
                nc.tensor.matmul(ps[:, 0:w], w2[:, ki, msl], h1[:, ki, 0:w],
                                 start=(ki == 0), stop=(ki == K2 - 1))
            # m-tile 2 carries the const-1 for h2[300] at partition 44
            bias = bc2 if mi == M2 - 1 else 0.0
            nc.scalar.activation(out=h2[:, mi, 0:w], in_=ps[:, 0:w],
                                 func=Relu, bias=bias)

        # ---- layer 3: [301 -> 100(pad 128)], bias via W3 row 300 --------
        # All K-slices use the full 128 partitions (tail rows are zeros):
        # sub-128-partition matmuls run ~100ns slower and stall the next MM.
        h3 = acts.tile([128, NCHUNK], BF16, tag="h3")
        ps = psum_mm.tile([128, NCHUNK], F32, tag="ps")
        for ki in range(K3):
            nc.tensor.matmul(ps[:, 0:w], w3[:, ki, :], h2[:, ki, 0:w],
                             start=(ki == 0), stop=(ki == K3 - 1))
        # bc3 carries the const-1 for h3[100] at partition 100
        nc.scalar.activation(out=h3[:, 0:w], in_=ps[:, 0:w], func=Relu,
                             bias=bc3)
        prev = (h3, w // 128, (c * NCHUNK + off) // 128, i % 2)

    emit_l4(True)

    for pool in (psum_bm, psum_mm, scratch, outs, acts, consts):
        pool.release()


def _pst_store(nc, scratch, outs, bm, out_blocks, g0, Gp, split_store=False,
               store_eng=None):
    """PST epilogue on one batch-major [128, Gp, 64] PSUM tile + store.

    bm holds pre-activations v (bias already included via the W4 const
    row). out63 = t - relu(t) * (1 - 1/denom): exact for t<=0 (the
    second term is 0) and equals t/denom for t>0.
    """
    Tanh = mybir.ActivationFunctionType.Tanh
    Sigm = mybir.ActivationFunctionType.Sigmoid

    o_sb = outs.tile([128, Gp, D_OUT], F32, tag="o")
    nc.scalar.activation(out=o_sb[:, :, 0:63], in_=bm[:, :, 0:63], func=Tanh)
    nc.scalar.activation(out=o_sb[:, :, 63:64], in_=bm[:, :, 63:64], func=Sigm)

    tv = o_sb[:, :, 0:63]  # tanh part [128, Gp, 63]
    rl = scratch.tile([128, Gp, 63], F32, tag="rl")
    nc.vector.tensor_scalar_max(rl, tv, 0.0)
    possum = scratch.tile([128, Gp], F32, tag="possum")
    nc.vector.reduce_sum(out=possum, in_=rl, axis=mybir.AxisListType.X)
    # denom: possum except 0 -> tiny (the t>0 correction is then 0 anyway)
    denom = scratch.tile([128, Gp], F32, tag="denom")
    nc.vector.tensor_scalar_max(denom, possum, 1e-30)
    recip = scratch.tile([128, Gp], F32, tag="recip")
    nc.vector.reciprocal(recip, denom)
    f = scratch.tile([128, Gp], F32, tag="f")  # 1 - 1/denom
    nc.vector.tensor_scalar(out=f, in0=recip, scalar1=-1.0, scalar2=1.0,
                            op0=mybir.AluOpType.mult,
                            op1=mybir.AluOpType.add)
    rlf = scratch.tile([128, Gp, 63], F32, tag="rlf")
    nc.vector.tensor_tensor(
        out=rlf, in0=rl, in1=f.unsqueeze(2).broadcast_to([128, Gp, 63]),
        op=mybir.AluOpType.mult)
    nc.vector.tensor_tensor(out=tv, in0=tv, in1=rlf,
                            op=mybir.AluOpType.subtract)
    if split_store:
        # final tile: parallel per-block stores to shrink the DMA tail
        engs = (nc.sync, nc.gpsimd, nc.scalar, nc.sync)
        for t in range(Gp):
            engs[t].dma_start(out=out_blocks[:, g0 + t, :], in_=o_sb[:, t, :])
    else:
        store_eng.dma_start(out=out_blocks[:, g0:g0 + Gp, :], in_=o_sb)


_PROG_CACHE = {}


def _build():
    if "nc" in _PROG_CACHE:
        return _PROG_CACHE["nc"]
    nc = bacc.Bacc("TRN2", target_bir_lowering=False, debug=False,
                   enable_asserts=False)
    aps = {
        "xT": nc.dram_tensor("xT", [D_IN, B], BF16, kind="ExternalInput").ap(),
        "W1": nc.dram_tensor("W1", [512, 512], BF16, kind="ExternalInput").ap(),
        "b1": nc.dram_tensor("b1", [512], F32, kind="ExternalInput").ap(),
        "W2": nc.dram_tensor("W2", [512, 384], BF16, kind="ExternalInput").ap(),
        "bc2": nc.dram_tensor("bc2", [128], F32, kind="ExternalInput").ap(),
        "W3": nc.dram_tensor("W3", [384, 128], BF16, kind="ExternalInput").ap(),
        "bc3": nc.dram_tensor("bc3", [128], F32, kind="ExternalInput").ap(),
        "W4": nc.dram_tensor("W4", [128, D_OUT], BF16, kind="ExternalInput").ap(),
        "out": nc.dram_tensor("out", [B, D_OUT], F32, kind="ExternalOutput").ap(),
    }
    with tile.TileContext(nc) as tc:
        _emit(tc, aps)
    nc.compile()
    _PROG_CACHE["nc"] = nc
    return nc


def kernel(state, W1, b1, W2, b2, W3, b3, W4, b4, _trace=False):
    import ml_dtypes

    nc = _build()
    BF = ml_dtypes.bfloat16

    def padbf(a, shape):
        out = np.zeros(shape, dtype=np.float32)
        a = np.asarray(a, dtype=np.float32)
        out[tuple(slice(0, s) for s in a.shape)] = a
        return out.astype(BF)

    # W2/W3/W4 get their bias as the row just past the real features; the
    # matching activation row is a constant 1 (b1p[500]=1 makes h1[500]=1,
    # bc2[44]=1 makes h2[300]=1 via L2 m-tile 2, bc3[100]=1 makes h3[100]=1).
    W2p = padbf(np.concatenate([np.asarray(W2, np.float32),
                                np.asarray(b2, np.float32)[None, :]], 0),
                (512, 384))
    W3p = padbf(np.concatenate([np.asarray(W3, np.float32),
                                np.asarray(b3, np.float32)[None, :]], 0),
                (384, 128))
    W4p = padbf(np.concatenate([np.asarray(W4, np.float32),
                                np.asarray(b4, np.float32)[None, :]], 0),
                (128, 64))
    b1p = np.zeros(512, dtype=np.float32)
    b1p[:500] = np.asarray(b1, np.float32)
    b1p[500] = 1.0
    bc2 = np.zeros(128, dtype=np.float32)
    bc2[44] = 1.0  # feature 300 = partition 44 of m-tile 2
    bc3 = np.zeros(128, dtype=np.float32)
    bc3[100] = 1.0

    weights = {
        "W1": padbf(np.asarray(W1, np.float32), (512, 512)),
        "b1": b1p, "W2": W2p, "bc2": bc2, "W3": W3p, "bc3": bc3, "W4": W4p,
    }
    state = np.asarray(state, dtype=np.float32)
    in_maps = []
    for i in range(N_CORES):
        shard = state[i * B:(i + 1) * B]
        in_maps.append({"xT": np.ascontiguousarray(shard.T).astype(BF), **weights})

    res = run_bass_kernel_spmd(nc, in_maps, core_ids=list(range(N_CORES)),
                               trace=_trace)
    full = np.concatenate([res.results[i]["out"] for i in range(N_CORES)], axis=0)
    if _trace:
        kernel.last_results = res
    return full


# revision 35
# speedup vs baseline: 1.0080x; 1.0080x over previous
"""Trainium2 Bass kernel for the DActor dense MLP.

Network (per row of `state`):
    h1 = relu(state @ W1 + b1)        # 512 -> 500
    h2 = relu(h1 @ W2 + b2)           # 500 -> 300
    h3 = relu(h2 @ W3 + b3)           # 300 -> 100
    v  = h3 @ W4 + b4                 # 100 -> 64
    t  = tanh(v[:, :63]); s = sigmoid(v[:, 63:])
    possum = sum(relu(t)); denom = possum == 0 ? 1 : possum
    out = concat(where(t > 0, t / denom, t), s)

Strategy: pure data parallel over 8 NeuronCores (8192 rows each).
Activations are kept feature-major ([feat, batch]) for L1-L3 so every
matmul uses the natural [fan_in, fan_out] weight tile as the stationary
operand and the activations as the 512-wide moving operand. Everything
runs in bf16 (matmul rate equals fp32r on the PE, but weight loads get
the fast-weight-load path and SBUF/DMA traffic halves; fp32 PSUM
accumulate keeps end-to-end error ~1e-3, well inside the 2e-2 gate.
fp8 DoubleRow was emulated end-to-end at 1.5-2.8e-2 — too close to or
over the gate — and rejected).

Biases for L2/L3/L4 ride inside the weight matmuls through constant-1
activation rows placed in the zero padding (h1[500]=1 with W2 row 500
= b2, h2[300]=1 with W3 row 300 = b3, h3[100]=1 with W4 row 100 = b4).
L4 is computed batch-major: each 128-row h3 block is the *stationary*
operand and W4 the 64-wide moving operand, so PSUM directly holds
[batch, 64] and no PE transpose / PSUM->SBUF copy is needed.

Edge-time details: ~32 short warmup matmuls on a zero tile run while
the initial DMAs land, so the PE's HAM clock gate reaches 2.4 GHz
before the first real matmul; initial weight/x DMAs are spread across
four engine queues (each dma_start costs its issuing engine ~0.7us of
descriptor generation, so x loads go to sync/gpsimd, stores to vector,
and the scalar engine keeps only ACT work); L3's 45-row K-tail is
padded to 128 partitions with zeros (sub-128-partition matmuls measure
~100ns slower and delay their successor); the PST epilogue runs
per-chunk and the last chunk's store is split four ways to shrink the
end-of-kernel DMA tail.
"""

import numpy as np

import concourse.bass as bass
import concourse.tile as tile
from concourse import bacc, mybir
from concourse.bass_utils import run_bass_kernel_spmd

N_CORES = 8
BATCH = 65536
B = BATCH // N_CORES  # 8192 rows per core
D_IN, H1, H2, H3, D_OUT = 512, 500, 300, 100, 64
NCHUNK = 512  # moving-operand width (= 1 PSUM bank of fp32)
N_CHUNKS = B // NCHUNK  # 16
G = 4  # 128-row blocks per chunk = per batch-major PSUM tile

F32 = mybir.dt.float32
BF16 = mybir.dt.bfloat16

K1, K2, K3 = 4, 4, 3  # 128-row K tiles per layer (L2 K=501 incl bias row)
M1, M2 = 4, 3  # output 128-tiles for L1 (500->512) / L2 (300->384)
K4 = 101  # L4 contraction: 100 features + const row


def _emit(tc: tile.TileContext, aps: dict):
    nc = tc.nc
    xT = aps["xT"]
    out = aps["out"]

    consts = tc.alloc_tile_pool(name="consts", bufs=1)
    acts = tc.alloc_tile_pool(name="acts", bufs=3)
    outs = tc.alloc_tile_pool(name="outs", bufs=3)
    scratch = tc.alloc_tile_pool(name="scratch", bufs=2)
    psum_mm = tc.alloc_tile_pool(name="psum_mm", bufs=7, space="PSUM")
    psum_bm = tc.alloc_tile_pool(name="psum_bm", bufs=1, space="PSUM")

    xT_v = xT.rearrange("(k p) b -> p k b", p=128)  # [128, 4, B]
    # out rows = 128*g + p  ->  [p, g, f] (g = global 128-row block index)
    out_blocks = out.rearrange("(g p) f -> p g f", p=128)

    # ---- PE warmup: short matmuls on a zeroed tile while DMAs land ------
    wtmp = consts.tile([128, 128], BF16)
    nc.gpsimd.memset(wtmp, 0)
    psw = psum_mm.tile([128, NCHUNK], F32, tag="ps")  # shares the ps ring
    # Dummy tanh first: pulls the ACT table load (~1.3us, sigmoid set
    # also serves relu/sigmoid) into the startup DMA window instead of
    # stalling the first mid-stream PST.
    tldw = scratch.tile([128, 8], F32, tag="tldw")
    nc.scalar.activation(out=tldw, in_=wtmp[:, 0:8],
                         func=mybir.ActivationFunctionType.Tanh)

    # ---- persistent constants -------------------------------------------
    # Host-padded weights, bf16. W2/W3/W4 carry their bias in the constant
    # row (500/300/100); padded rows/cols are zero.
    w1 = consts.tile([128, K1, 512], BF16)
    w2 = consts.tile([128, K2, 384], BF16)
    w3 = consts.tile([128, K3, 128], BF16)
    w4 = consts.tile([128, D_OUT], BF16)
    b1 = consts.tile([128, M1], F32)
    bc2 = consts.tile([128, 1], F32)
    bc3 = consts.tile([128, 1], F32)

    # Only sync (SP), gpsimd, and scalar queues can issue DMAs; keep the
    # scalar engine's queue for startup only (ACT is the second-busiest).
    # Startup DMAs are ordered so w1-k0 and x0-k0 land first on separate
    # queues (they gate the first matmul; each trigger costs its engine
    # ~0.7us of descriptor generation).
    w1_v = aps["W1"].rearrange("(k p) m -> p k m", p=128)
    x_tiles = {}

    def load_x(c, engines=None):
        t = acts.tile([128, K1, NCHUNK], BF16, tag="x")
        cs = slice(c * NCHUNK, (c + 1) * NCHUNK)
        if engines:  # startup: k-tiles on separate queues, land in parallel
            for ki in range(K1):
                engines[ki % len(engines)].dma_start(out=t[:, ki, :],
                                                     in_=xT_v[:, ki, cs])
        else:
            # steady state: one trigger per chunk (a dma_start costs its
            # engine ~0.7us of descriptor generation; the sequencers also
            # carry the semaphore bookkeeping for the PE stream)
            eng = nc.sync if c % 2 == 0 else nc.gpsimd
            eng.dma_start(out=t, in_=xT_v[:, :, cs])
        x_tiles[c] = t

    # Chunk 0 is processed as two 256-col halves (see `specs` below), and
    # its x/w1-k0 loads arrive as 256-col pieces on separate queues, so the
    # first matmul is gated by 64KB transfers instead of 256KB. ~30 warmup
    # matmuls cover the HAM clock ramp so real matmuls start warm (~5us)
    # instead of waiting for full-tile transfers (~12us). All pieces keep
    # the full 128-partition shape (64-partition DMAs flip the chip into a
    # 5/6-clock state -- measured, mechanism unknown).
    x0 = acts.tile([128, K1, NCHUNK], BF16, tag="x")
    x_tiles[0] = x0
    A, Bc = slice(0, 256), slice(256, 512)
    nc.sync.dma_start(out=w1[:, 0, A], in_=w1_v[:, 0, A])
    nc.gpsimd.dma_start(out=x0[:, 0, A], in_=xT_v[:, 0, A])
    nc.scalar.dma_start(out=w1[:, 0, Bc], in_=w1_v[:, 0, Bc])
    nc.sync.dma_start(out=x0[:, 1, A], in_=xT_v[:, 1, A])
    nc.gpsimd.dma_start(out=b1, in_=aps["b1"].rearrange("(m p) -> p m", p=128))
    nc.scalar.dma_start(out=x0[:, 1, Bc], in_=xT_v[:, 1, Bc])
    nc.sync.dma_start(out=w1[:, 2, :], in_=w1_v[:, 2, :])
    nc.gpsimd.dma_start(out=w1[:, 1, :], in_=w1_v[:, 1, :])
    nc.scalar.dma_start(out=w1[:, 3, :], in_=w1_v[:, 3, :])
    nc.sync.dma_start(out=x0[:, 3, A], in_=xT_v[:, 3, A])
    nc.gpsimd.dma_start(out=x0[:, 2, A], in_=xT_v[:, 2, A])
    nc.scalar.dma_start(out=x0[:, 3, Bc], in_=xT_v[:, 3, Bc])
    nc.gpsimd.dma_start(out=x0[:, 0, Bc], in_=xT_v[:, 0, Bc])
    nc.gpsimd.dma_start(out=x0[:, 2, Bc], in_=xT_v[:, 2, Bc])

    for _ in range(30):
        nc.tensor.matmul(psw[:, 0:128], wtmp, wtmp, start=True, stop=True)

    w2_v = aps["W2"].rearrange("(k p) m -> p k m", p=128)
    nc.sync.dma_start(out=w2[:, 0:2, :], in_=w2_v[:, 0:2, :])
    nc.scalar.dma_start(out=w2[:, 2:4, :], in_=w2_v[:, 2:4, :])
    nc.gpsimd.dma_start(out=bc2, in_=aps["bc2"].rearrange("(m o) -> m o", o=1))
    nc.gpsimd.dma_start(out=bc3, in_=aps["bc3"].rearrange("(m o) -> m o", o=1))
    nc.scalar.dma_start(out=w3, in_=aps["W3"].rearrange("(k p) m -> p k m", p=128))
    nc.gpsimd.dma_start(out=w4, in_=aps["W4"])
    load_x(1, [nc.sync, nc.gpsimd, nc.sync, nc.gpsimd])

    Relu = mybir.ActivationFunctionType.Relu
    prev = None  # (h3 tile, G blocks, global block base, store engine parity)

    # First and last 512-col chunks are processed as two 256-col halves:
    # the first so compute starts as soon as its 64KB pieces land, the last
    # so the end-of-kernel serial PST chain and final store are half-sized.
    specs = [(0, 0, 256), (0, 256, 256)]
    specs += [(c, 0, NCHUNK) for c in range(1, N_CHUNKS - 1)]
    specs += [(N_CHUNKS - 1, 0, 256), (N_CHUNKS - 1, 256, 256)]

    def emit_l4(is_last):
        # L4 for the previous chunk, batch-major: each 128-row h3 block is
        # the stationary operand, W4 (64 cols, bias in row 100) the moving
        # one. Emitted after the next chunk's L1 matmuls so the PE never
        # waits on the ACT-produced h3.
        if prev is None:
            return
        h3p, Gp, g0, par = prev
        bm = psum_bm.tile([128, Gp, D_OUT], F32, tag="bm")
        for bb in range(Gp):
            nc.tensor.matmul(
                bm[:, bb, :],
                h3p[:K4, bb * 128:(bb + 1) * 128],
                w4[:K4, :],
                start=True, stop=True,
            )
        # store on the opposite queue of the chunk's x load
        _pst_store(nc, scratch, outs, bm, out_blocks, g0, Gp,
                   split_store=is_last,
                   store_eng=nc.gpsimd if (g0 // 4) % 2 == 0 else nc.sync)

    for i, (c, off, w) in enumerate(specs):
        if c + 2 < N_CHUNKS and off == 0:
            load_x(c + 2)
        x_sb = x_tiles[c]
        if off + w == NCHUNK:
            x_tiles.pop(c)
        ws = slice(off, off + w)

        # ---- layer 1: [512 -> 500(pad 512)], bias via ACT ---------------
        # m-tile 3's relu+bias runs on DVE (in parallel with ACT doing
        # m2) so h1-m3 is ready before L2's k3 matmul needs it.
        h1 = acts.tile([128, K2, NCHUNK], BF16, tag="h1")
        for mi in range(M1):
            ps = psum_mm.tile([128, NCHUNK], F32, tag="ps")
            msl = slice(mi * 128, (mi + 1) * 128)
            for ki in range(K1):
                nc.tensor.matmul(ps[:, 0:w], w1[:, ki, msl], x_sb[:, ki, ws],
                                 start=(ki == 0), stop=(ki == K1 - 1))
            if mi == M1 - 1:
                nc.vector.tensor_scalar(out=h1[:, mi, 0:w], in0=ps[:, 0:w],
                                        scalar1=b1[:, mi:mi + 1], scalar2=0.0,
                                        op0=mybir.AluOpType.add,
                                        op1=mybir.AluOpType.max)
            else:
                nc.scalar.activation(out=h1[:, mi, 0:w], in_=ps[:, 0:w],
                                     func=Relu, bias=b1[:, mi:mi + 1])
        emit_l4(False)

        # ---- layer 2: [501 -> 300(pad 384)], bias via W2 row 500 --------
        h2 = acts.tile([128, K3, NCHUNK], BF16, tag="h2")
        for mi in range(M2):
            ps = psum_mm.tile([128, NCHUNK], F32, tag="ps")
            msl = slice(mi * 128, (mi + 1) * 128)
            for ki in range(K2):
                nc.tensor.matmul(ps[:, 0:w], w2[:, ki, msl], h1[:, ki, 0:w],
                                 start=(ki == 0), stop=(ki == K2 - 1))
            # m-tile 2 carries the const-1 for h2[300] at partition 44
            bias = bc2 if mi == M2 - 1 else 0.0
            nc.scalar.activation(out=h2[:, mi, 0:w], in_=ps[:, 0:w],
                                 func=Relu, bias=bias)

        # ---- layer 3: [301 -> 100(pad 128)], bias via W3 row 300 --------
        # All K-slices use the full 128 partitions (tail rows are zeros):
        # sub-128-partition matmuls run ~100ns slower and stall the next MM.
        h3 = acts.tile([128, NCHUNK], BF16, tag="h3")
        ps = psum_mm.tile([128, NCHUNK], F32, tag="ps")
        for ki in range(K3):
            nc.tensor.matmul(ps[:, 0:w], w3[:, ki, :], h2[:, ki, 0:w],
                             start=(ki == 0), stop=(ki == K3 - 1))
        # bc3 carries the const-1 for h3[100] at partition 100
        nc.scalar.activation(out=h3[:, 0:w], in_=ps[:, 0:w], func=Relu,
                             bias=bc3)
        prev = (h3, w // 128, (c * NCHUNK + off) // 128, i % 2)

    emit_l4(True)

    for pool in (psum_bm, psum_mm, scratch, outs, acts, consts):
        pool.release()


def _pst_store(nc, scratch, outs, bm, out_blocks, g0, Gp, split_store=False,
               store_eng=None):
    """PST epilogue on one batch-major [128, Gp, 64] PSUM tile + store.

    bm holds pre-activations v (bias already included via the W4 const
    row). out63 = t - relu(t) * (1 - 1/denom): exact for t<=0 (the
    second term is 0) and equals t/denom for t>0.
    """
    Tanh = mybir.ActivationFunctionType.Tanh
    Sigm = mybir.ActivationFunctionType.Sigmoid

    o_sb = outs.tile([128, Gp, D_OUT], F32, tag="o")
    nc.scalar.activation(out=o_sb[:, :, 0:63], in_=bm[:, :, 0:63], func=Tanh)
    nc.scalar.activation(out=o_sb[:, :, 63:64], in_=bm[:, :, 63:64], func=Sigm)

    tv = o_sb[:, :, 0:63]  # tanh part [128, Gp, 63]
    rl = scratch.tile([128, Gp, 63], F32, tag="rl")
    nc.vector.tensor_scalar_max(rl, tv, 0.0)
    possum = scratch.tile([128, Gp], F32, tag="possum")
    nc.vector.reduce_sum(out=possum, in_=rl, axis=mybir.AxisListType.X)
    # denom: possum except 0 -> tiny (the t>0 correction is then 0 anyway)
    denom = scratch.tile([128, Gp], F32, tag="denom")
    nc.vector.tensor_scalar_max(denom, possum, 1e-30)
    recip = scratch.tile([128, Gp], F32, tag="recip")
    nc.vector.reciprocal(recip, denom)
    f = scratch.tile([128, Gp], F32, tag="f")  # 1 - 1/denom
    nc.vector.tensor_scalar(out=f, in0=recip, scalar1=-1.0, scalar2=1.0,
                            op0=mybir.AluOpType.mult,
                            op1=mybir.AluOpType.add)
    rlf = scratch.tile([128, Gp, 63], F32, tag="rlf")
    nc.vector.tensor_tensor(
        out=rlf, in0=rl, in1=f.unsqueeze(2).broadcast_to([128, Gp, 63]),
        op=mybir.AluOpType.mult)
    nc.vector.tensor_tensor(out=tv, in0=tv, in1=rlf,
                            op=mybir.AluOpType.subtract)
    if split_store:
        # final tile: parallel per-block stores to shrink the DMA tail
        engs = (nc.sync, nc.gpsimd, nc.scalar, nc.sync)
        for t in range(Gp):
            engs[t].dma_start(out=out_blocks[:, g0 + t, :], in_=o_sb[:, t, :])
    else:
        store_eng.dma_start(out=out_blocks[:, g0:g0 + Gp, :], in_=o_sb)


_PROG_CACHE = {}


def _build():
    if "nc" in _PROG_CACHE:
        return _PROG_CACHE["nc"]
    nc = bacc.Bacc("TRN2", target_bir_lowering=False, debug=False,
                   enable_asserts=False)
    aps = {
        "xT": nc.dram_tensor("xT", [D_IN, B], BF16, kind="ExternalInput").ap(),
        "W1": nc.dram_tensor("W1", [512, 512], BF16, kind="ExternalInput").ap(),
        "b1": nc.dram_tensor("b1", [512], F32, kind="ExternalInput").ap(),
        "W2": nc.dram_tensor("W2", [512, 384], BF16, kind="ExternalInput").ap(),
        "bc2": nc.dram_tensor("bc2", [128], F32, kind="ExternalInput").ap(),
        "W3": nc.dram_tensor("W3", [384, 128], BF16, kind="ExternalInput").ap(),
        "bc3": nc.dram_tensor("bc3", [128], F32, kind="ExternalInput").ap(),
        "W4": nc.dram_tensor("W4", [128, D_OUT], BF16, kind="ExternalInput").ap(),
        "out": nc.dram_tensor("out", [B, D_OUT], F32, kind="ExternalOutput").ap(),
    }
    with tile.TileContext(nc) as tc:
        _emit(tc, aps)
    nc.compile()
    _PROG_CACHE["nc"] = nc
    return nc


def kernel(state, W1, b1, W2, b2, W3, b3, W4, b4, _trace=False):
    import ml_dtypes

    nc = _build()
    BF = ml_dtypes.bfloat16

    def padbf(a, shape):
        out = np.zeros(shape, dtype=np.float32)
        a = np.asarray(a, dtype=np.float32)
        out[tuple(slice(0, s) for s in a.shape)] = a
        return out.astype(BF)

    # W2/W3/W4 get their bias as the row just past the real features; the
    # matching activation row is a constant 1 (b1p[500]=1 makes h1[500]=1,
    # bc2[44]=1 makes h2[300]=1 via L2 m-tile 2, bc3[100]=1 makes h3[100]=1).
    W2p = padbf(np.concatenate([np.asarray(W2, np.float32),
                                np.asarray(b2, np.float32)[None, :]], 0),
                (512, 384))
    W3p = padbf(np.concatenate([np.asarray(W3, np.float32),
                                np.asarray(b3, np.float32)[None, :]], 0),
                (384, 128))
    W4p = padbf(np.concatenate([np.asarray(W4, np.float32),
                                np.asarray(b4, np.float32)[None, :]], 0),
                (128, 64))
    b1p = np.zeros(512, dtype=np.float32)
    b1p[:500] = np.asarray(b1, np.float32)
    b1p[500] = 1.0
    bc2 = np.zeros(128, dtype=np.float32)
    bc2[44] = 1.0  # feature 300 = partition 44 of m-tile 2
    bc3 = np.zeros(128, dtype=np.float32)
    bc3[100] = 1.0

    weights = {
        "W1": padbf(np.asarray(W1, np.float32), (512, 512)),
        "b1": b1p, "W2": W2p, "bc2": bc2, "W3": W3p, "bc3": bc3, "W4": W4p,
    }
    state = np.asarray(state, dtype=np.float32)
    in_maps = []
    for i in range(N_CORES):
        shard = state[i * B:(i + 1) * B]
        in_maps.append({"xT": np.ascontiguousarray(shard.T).astype(BF), **weights})

    res = run_bass_kernel_spmd(nc, in_maps, core_ids=list(range(N_CORES)),
                               trace=_trace)
    full = np.concatenate([res.results[i]["out"] for i in range(N_CORES)], axis=0)
    if _trace:
        kernel.last_results = res
    return full
